# revision 27
# baseline (speedup 1.0000x reference)
"""GQA kernel for trn2: 8 NeuronCores, SPMD (b in {0,1} x 4 head-groups).

Per core (b, hg): 8 q-heads (8hg..8hg+7), 2 kv-heads (2hg, 2hg+1).
c-major software pipeline: per 512-wide q chunk c emit
  proj(c) -> RoPE -> V-build -> attention(c) with outproj(c-1) interleaved
so PE never sees a phase barrier (keeps HAM un-throttled).
V blocks are [v0|ones64|v1] so the attn@V matmul also produces the softmax
denominator replicated across 64 partitions (normalize = recip + 2 muls,
no partition broadcast). f32r proj, bf16 scores/attn@V, fp16 outproj.
Host preps transposed/permuted weights; partial outputs summed on host
(row-parallel Wo all-reduce done during unshard).
"""
import numpy as np
import ml_dtypes
import concourse.bass as bass
import concourse.mybir as mybir
from concourse import tile, bacc
from concourse.bass_utils import run_bass_kernel_spmd

B, S, D = 2, 2048, 2048
H, KVH, DH = 32, 8, 64
SCALE = DH ** -0.5
SC = 4          # Sq chunks of 512
KD = 16         # D contraction chunks of 128
NJ = 16         # Sk blocks of 128
F32R = mybir.dt.float32r
F32 = mybir.dt.float32
BF16 = mybir.dt.bfloat16
F16 = mybir.dt.float16

_cache = {}


def build():
    nc = bacc.Bacc('TRN2', target_bir_lowering=False, debug=False, num_devices=8)
    xT_p = nc.declare_dram_parameter('xT', [D, S], F32R, isOutput=False)
    WT_p = nc.declare_dram_parameter('WT', [D, 768], F32R, isOutput=False)
    WoT_p = nc.declare_dram_parameter('WoT', [512, D], F16, isOutput=False)
    cos4_p = nc.declare_dram_parameter('cos4', [128, S], F16, isOutput=False)
    sin4_p = nc.declare_dram_parameter('sin4', [128, S], F16, isOutput=False)
    mask_p = nc.declare_dram_parameter('mask', [128, 4 * 1024], BF16, isOutput=False)
    ident_p = nc.declare_dram_parameter('ident', [128, 128], F32, isOutput=False)
    out_p = nc.declare_dram_parameter('out', [S, D], F32, isOutput=True)

    with tile.TileContext(nc) as tc:
        with tc.tile_pool(name='w', bufs=1) as wpool, \
             tc.tile_pool(name='x', bufs=20) as xpool, \
             tc.tile_pool(name='q', bufs=8) as qpool, \
             tc.tile_pool(name='ao', bufs=8) as aopool, \
             tc.tile_pool(name='v', bufs=2) as vpool, \
             tc.tile_pool(name='t', bufs=2) as tpool, \
             tc.tile_pool(name='at', bufs=4) as atpool, \
             tc.tile_pool(name='s', bufs=2) as spool, \
             tc.tile_pool(name='o', bufs=2) as opool, \
             tc.tile_pool(name='acc', bufs=2, space='PSUM') as accp, \
             tc.tile_pool(name='sc', bufs=2, space='PSUM') as scp, \
             tc.tile_pool(name='po', bufs=1, space='PSUM') as pop:

            WT = wpool.tile([128, KD * 768], F32R, tag='WT')
            cos4 = wpool.tile([128, S], F16, tag='cos4')
            sin4 = wpool.tile([128, S], F16, tag='sin4')
            masks = wpool.tile([128, 4 * 1024], BF16, tag='masks')
            ident = wpool.tile([128, 128], F32, tag='ident')
            k4 = wpool.tile([128, S], BF16, tag='k4')
            kswap = wpool.tile([128, S], BF16, tag='kswap')
            V = wpool.tile([128, NJ * 320], BF16, tag='V')
            WoT = wpool.tile([128, 4 * D], F16, tag='WoT')

            # startup: interleave WT and first-chunk x DMAs so kd=0 lands fast
            xts0 = []
            for kd in range(KD):
                nc.sync.dma_start(out=WT[:, kd * 768:(kd + 1) * 768],
                                  in_=WT_p[128 * kd:128 * (kd + 1), :])
                xt = xpool.tile([128, 512], F32R, tag='xt')
                nc.sync.dma_start(out=xt[:], in_=xT_p[128 * kd:128 * (kd + 1), 0:512])
                xts0.append(xt)
            nc.sync.dma_start(out=cos4[:], in_=cos4_p[:])
            nc.sync.dma_start(out=sin4[:], in_=sin4_p[:])
            nc.sync.dma_start(out=masks[:], in_=mask_p[:])
            nc.sync.dma_start(out=ident[:], in_=ident_p[:])
            nc.vector.memset(V[:], 1.0)

            aout_c = {}   # (c, hp) -> per-chunk fp16 attention-output tile

            def outproj_unit(cc, sb, dg):
                # one (s-block, 1024-wide D group): 8 matmuls + 2 evacs + 1 DMA
                ost = opool.tile([128, 1024], F32, tag='ost')
                for i, dc in enumerate((2 * dg, 2 * dg + 1)):
                    po = accp.tile([128, 512], F32, tag='acc', name=f'po{sb}_{dc}')
                    for hc in range(4):
                        nc.tensor.matmul(po[:], aout_c[(cc, hc)][:, 128 * (sb - 4 * cc):128 * (sb - 4 * cc) + 128],
                                         WoT[:, hc * D + 512 * dc: hc * D + 512 * (dc + 1)],
                                         start=(hc == 0), stop=(hc == 3))
                    if dc % 2 == 0:
                        nc.scalar.copy(ost[:, 512 * i:512 * (i + 1)], po[:])
                    else:
                        nc.vector.tensor_copy(ost[:, 512 * i:512 * (i + 1)], po[:])
                nc.sync.dma_start(out=out_p[128 * sb:128 * (sb + 1), 1024 * dg:1024 * (dg + 1)],
                                  in_=ost[:])

            qc_c = {}    # chunk -> list of 4 q tiles
            xts_c = {0: xts0}

            def emit_xt_dmas(cc):
                if cc in xts_c or cc >= SC:
                    return
                lst = []
                for kd in range(KD):
                    xt = xpool.tile([128, 512], F32R, tag='xt')
                    nc.sync.dma_start(out=xt[:],
                                      in_=xT_p[128 * kd:128 * (kd + 1), 512 * cc:512 * (cc + 1)])
                    lst.append(xt)
                xts_c[cc] = lst

            def proj_unit(cc, m):
                # one projection output block (128 rows) + its epilogue
                cs = slice(512 * cc, 512 * (cc + 1))
                xts = xts_c[cc]
                ps = accp.tile([128, 512], F32, tag='acc', name=f'ps{cc}_{m}')
                for kd in range(KD):
                    nc.tensor.matmul(ps[:], WT[:, kd * 768 + 128 * m: kd * 768 + 128 * (m + 1)],
                                     xts[kd][:], start=(kd == 0), stop=(kd == KD - 1))
                if m < 5:
                    # RoPE: out = ps*cos4 + swap32(ps)*sin4 (sign baked in sin4)
                    t1 = tpool.tile([128, 512], F32, tag='t1')
                    t2 = tpool.tile([128, 512], F32, tag='t2')
                    nc.vector.tensor_mul(t1[:], ps[:], cos4[:, cs])
                    for g in range(2):
                        b0 = 64 * g
                        nc.vector.tensor_mul(t2[b0:b0 + 32, :], ps[b0 + 32:b0 + 64, :],
                                             sin4[b0:b0 + 32, cs])
                        nc.vector.tensor_mul(t2[b0 + 32:b0 + 64, :], ps[b0:b0 + 32, :],
                                             sin4[b0 + 32:b0 + 64, cs])
                    tgt = qc_c[cc][m][:] if m < 4 else k4[:, cs]
                    nc.vector.tensor_add(tgt, t1[:], t2[:])
                    if m == 4:
                        # kT replication: rep0=[kv0|kv0], rep1=[kv1|kv1]
                        nc.vector.tensor_add(kswap[0:64, cs], t1[64:128, :], t2[64:128, :])
                        nc.vector.tensor_add(kswap[64:128, cs], t1[0:64, :], t2[0:64, :])
                else:
                    vT = vpool.tile([128, 512], F32, tag='vT')
                    nc.scalar.copy(vT[:], ps[:])
                    # V natural (bf16): per j [ones|v0|ones|v1|ones] (5x64)
                    for j in range(4 * cc, 4 * cc + 4):
                        jj = j - 4 * cc
                        pt = accp.tile([128, 128], F32, tag='acc', name=f'pt{j}')
                        nc.tensor.transpose(pt[:], vT[:, 128 * jj:128 * jj + 128], ident[:])
                        nc.vector.tensor_copy(V[:, 320 * j + 64:320 * j + 128], pt[:, 0:64])
                        nc.vector.tensor_copy(V[:, 320 * j + 192:320 * j + 256], pt[:, 64:128])

            def emit_proj(cc):
                emit_xt_dmas(cc)
                qc_c[cc] = [qpool.tile([128, 512], BF16, tag='qc', name=f'qc{cc}_{m}')
                            for m in range(4)]
                for m in (4, 5, 0, 1, 2, 3):
                    proj_unit(cc, m)

            emit_proj(0)
            for hc in range(4):
                nc.sync.dma_start(out=WoT[:, hc * D:(hc + 1) * D],
                                  in_=WoT_p[128 * hc:128 * (hc + 1), :])

            for c in range(SC):
                # filler units spread through attention(c):
                # outproj of chunk c-1, then projection blocks of chunk c+1
                fillers = []
                if c >= 1:
                    for sb in range(4 * (c - 1), 4 * c):
                        for dg in range(2):
                            fillers.append((outproj_unit, (c - 1, sb, dg)))
                if c + 1 < SC:
                    emit_xt_dmas(c + 1)
                    qc_c[c + 1] = [qpool.tile([128, 512], BF16, tag='qc', name=f'qc{c+1}_{m}')
                                   for m in range(4)]
                    for m in (4, 5, 0, 1, 2, 3):
                        fillers.append((proj_unit, (c + 1, m)))
                nj = 4 * c + 4
                npoints = 4 * (nj - 1)
                stride = max(1, npoints // (len(fillers) + 1)) if fillers else 0
                point = 0
                qc = qc_c[c]

                for hp in range(4):
                    kv = hp // 2
                    oAB = pop.tile([128, 1024], F32, tag='oab')
                    atas = {}

                    def scores(j):
                        sct = scp.tile([128, 1024], F32, tag='sc', name=f'sc{hp}_{c}_{j}')
                        if kv == 0:
                            kA = k4[0:64, 128 * j:128 * (j + 1)]
                            kB = kswap[64:128, 128 * j:128 * (j + 1)]
                        else:
                            kA = kswap[0:64, 128 * j:128 * (j + 1)]
                            kB = k4[64:128, 128 * j:128 * (j + 1)]
                        nc.tensor.matmul(sct[:, 0:512], kA, qc[hp][0:64, :], start=True, stop=True)
                        nc.tensor.matmul(sct[:, 512:1024], kB, qc[hp][64:128, :], start=True, stop=True)
                        ata = atpool.tile([128, 1024], BF16, tag='at', name=f'at{hp}_{c}_{j}')
                        nc.scalar.activation(ata[:], sct[:], mybir.ActivationFunctionType.Exp,
                                             scale=SCALE)
                        d = j - 4 * c
                        if 0 <= d <= 3:
                            nc.vector.tensor_mul(ata[:, 0:512], ata[:, 0:512],
                                                 masks[:, 1024 * d:1024 * d + 512])
                            nc.gpsimd.tensor_mul(ata[:, 512:1024], ata[:, 512:1024],
                                                 masks[:, 1024 * d + 512:1024 * (d + 1)])
                        atas[j] = ata

                    def attnv(j):
                        # A slice [v|ones] -> rows 0:64 data, 64:128 denom
                        # B slice [ones|v] -> rows 0:64 denom, 64:128 data
                        ata = atas.pop(j)
                        base = 320 * j + 128 * kv
                        nc.tensor.matmul(oAB[:, 0:512], V[:, base + 64:base + 192],
                                         ata[:, 0:512], start=(j == 0), stop=(j == nj - 1))
                        nc.tensor.matmul(oAB[:, 512:1024], V[:, base:base + 128],
                                         ata[:, 512:1024], start=(j == 0), stop=(j == nj - 1))

                    scores(0)
                    for j in range(1, nj):
                        scores(j)
                        attnv(j - 1)
                        point += 1
                        if fillers and stride and point % stride == 0:
                            fn, args = fillers.pop(0)
                            fn(*args)
                    attnv(nj - 1)

                    # normalize: evacuate PSUM into base-0 data/denom tiles,
                    # approx recip on the denom tile, two base-aligned muls
                    dsw = spool.tile([64, 1024], F32, tag='dsw')
                    nc.scalar.copy(dsw[:, 0:512], oAB[64:128, 0:512])
                    nc.scalar.copy(dsw[:, 512:1024], oAB[0:64, 512:1024])
                    sbD = spool.tile([64, 1024], F32, tag='sbd')
                    nc.vector.tensor_copy(sbD[:, 0:512], oAB[0:64, 0:512])
                    nc.vector.tensor_copy(sbD[:, 512:1024], oAB[64:128, 512:1024])
                    rr = spool.tile([64, 1024], F32, tag='rr')
                    nc.vector.reciprocal_approx_fast(rr[:], dsw[:])
                    ao = aopool.tile([128, 512], F16, tag='ao', name=f'ao{c}_{hp}')
                    aout_c[(c, hp)] = ao
                    nc.vector.tensor_mul(ao[0:64, :], sbD[:, 0:512], rr[:, 0:512])
                    nc.vector.tensor_mul(ao[64:128, :], sbD[:, 512:1024], rr[:, 512:1024])

                for fn, args in fillers:
                    fn(*args)

            for sb in range(12, 16):
                for dg in range(2):
                    outproj_unit(3, sb, dg)
    nc.compile()
    return nc


_PERM = np.concatenate([np.arange(0, DH, 2), np.arange(1, DH, 2)])


def _prep_core(x, Wq, Wk, Wv, Wo, cos, sin, b, hg):
    xT = np.ascontiguousarray(x[b].T.astype(np.float32))
    # q heads 8hg..8hg+7 permuted, kv heads 2hg,2hg+1 (k permuted, v natural)
    wq = Wq.reshape(H, DH, D)[8 * hg:8 * hg + 8][:, _PERM, :].reshape(512, D)
    wk = Wk.reshape(KVH, DH, D)[2 * hg:2 * hg + 2][:, _PERM, :].reshape(128, D)
    wv = Wv.reshape(KVH, DH, D)[2 * hg:2 * hg + 2].reshape(128, D)
    WT = np.ascontiguousarray(np.concatenate([wq, wk, wv], 0).T.astype(np.float32))
    WoT = np.ascontiguousarray(Wo[:, 512 * hg:512 * (hg + 1)].T.astype(np.float16))
    cosT = np.ascontiguousarray(cos.T.astype(np.float32))          # (32, S)
    sinT = np.ascontiguousarray(sin.T.astype(np.float32))
    cos4 = np.tile(cosT, (4, 1)).astype(np.float16)
    sin4 = np.concatenate([-sinT, sinT, -sinT, sinT], 0).astype(np.float16)
    mask = np.zeros((128, 4 * 1024), dtype=np.float64)
    for dd in range(4):
        blk = (128 * dd + np.arange(128)[:, None]) <= np.arange(512)[None, :]
        mask[:, 1024 * dd:1024 * dd + 512] = blk
        mask[:, 1024 * dd + 512:1024 * (dd + 1)] = blk
    return {'xT': xT, 'WT': WT, 'WoT': WoT, 'cos4': cos4, 'sin4': sin4,
            'mask': mask.astype(ml_dtypes.bfloat16),
            'ident': np.eye(128, dtype=np.float32)}


def _run(inputs, trace=False, tmpdir=None):
    if 'nc' not in _cache:
        _cache['nc'] = build()
    in_maps = [_prep_core(inputs['x'], inputs['Wq'], inputs['Wk'], inputs['Wv'],
                          inputs['Wo'], inputs['cos'], inputs['sin'], c // 4, c % 4)
               for c in range(8)]
    res = run_bass_kernel_spmd(_cache['nc'], in_maps, core_ids=list(range(8)),
                               trace=trace, tmpdir=tmpdir)
    parts = [res.results[c]['out'] for c in range(8)]
    out = np.stack([parts[0] + parts[1] + parts[2] + parts[3],
                    parts[4] + parts[5] + parts[6] + parts[7]], 0)
    return out.astype(np.float32), res


def kernel(**inputs):
    out, _ = _run(inputs, trace=False)
    return out


# revision 29
# speedup vs baseline: 1.0736x; 1.0736x over previous
"""GQA kernel for trn2: 8 NeuronCores, SPMD (b in {0,1} x 4 head-groups).

Per core (b, hg): 8 q-heads (8hg..8hg+7), 2 kv-heads (2hg, 2hg+1).
c-major software pipeline: per 512-wide q chunk c emit
  proj(c) -> RoPE -> V-build -> attention(c) with outproj(c-1) interleaved
so PE never sees a phase barrier (keeps HAM un-throttled).
V blocks are [v0|ones64|v1] so the attn@V matmul also produces the softmax
denominator replicated across 64 partitions (normalize = recip + 2 muls,
no partition broadcast). f32r proj, bf16 scores/attn@V, fp16 outproj.
Host preps transposed/permuted weights; partial outputs summed on host
(row-parallel Wo all-reduce done during unshard).
"""
import numpy as np
import ml_dtypes
import concourse.bass as bass
import concourse.mybir as mybir
from concourse import tile, bacc
from concourse.bass_utils import run_bass_kernel_spmd

B, S, D = 2, 2048, 2048
H, KVH, DH = 32, 8, 64
SCALE = DH ** -0.5
SC = 4          # Sq chunks of 512
KD = 16         # D contraction chunks of 128
NJ = 16         # Sk blocks of 128
F32R = mybir.dt.float32r
F32 = mybir.dt.float32
BF16 = mybir.dt.bfloat16
F16 = mybir.dt.float16

_cache = {}


def build():
    nc = bacc.Bacc('TRN2', target_bir_lowering=False, debug=False, num_devices=8)
    xT_p = nc.declare_dram_parameter('xT', [D, S], F32R, isOutput=False)
    WT_p = nc.declare_dram_parameter('WT', [D, 768], F32R, isOutput=False)
    WoT_p = nc.declare_dram_parameter('WoT', [512, D], F16, isOutput=False)
    cos4_p = nc.declare_dram_parameter('cos4', [128, S], F16, isOutput=False)
    sin4_p = nc.declare_dram_parameter('sin4', [128, S], F16, isOutput=False)
    mask_p = nc.declare_dram_parameter('mask', [128, 4 * 1024], BF16, isOutput=False)
    ident_p = nc.declare_dram_parameter('ident', [128, 128], F32, isOutput=False)
    out_p = nc.declare_dram_parameter('out', [S, D], F32, isOutput=True)

    with tile.TileContext(nc) as tc:
        with tc.tile_pool(name='w', bufs=1) as wpool, \
             tc.tile_pool(name='x', bufs=20) as xpool, \
             tc.tile_pool(name='q', bufs=8) as qpool, \
             tc.tile_pool(name='ao', bufs=8) as aopool, \
             tc.tile_pool(name='v', bufs=2) as vpool, \
             tc.tile_pool(name='t', bufs=2) as tpool, \
             tc.tile_pool(name='at', bufs=4) as atpool, \
             tc.tile_pool(name='s', bufs=2) as spool, \
             tc.tile_pool(name='o', bufs=2) as opool, \
             tc.tile_pool(name='acc', bufs=2, space='PSUM') as accp, \
             tc.tile_pool(name='sc', bufs=2, space='PSUM') as scp, \
             tc.tile_pool(name='po', bufs=1, space='PSUM') as pop:

            WT = wpool.tile([128, KD * 768], F32R, tag='WT')
            cos4 = wpool.tile([128, S], F16, tag='cos4')
            sin4 = wpool.tile([128, S], F16, tag='sin4')
            masks = wpool.tile([128, 4 * 1024], BF16, tag='masks')
            ident = wpool.tile([128, 128], F32, tag='ident')
            k4 = wpool.tile([128, S], BF16, tag='k4')
            kswap = wpool.tile([128, S], BF16, tag='kswap')
            V = wpool.tile([128, NJ * 320], BF16, tag='V')
            WoT = wpool.tile([128, 4 * D], F16, tag='WoT')

            # startup: interleave WT and first-chunk x DMAs so kd=0 lands fast
            xts0 = []
            for kd in range(KD):
                nc.sync.dma_start(out=WT[:, kd * 768:(kd + 1) * 768],
                                  in_=WT_p[128 * kd:128 * (kd + 1), :])
                xt = xpool.tile([128, 512], F32R, tag='xt')
                nc.sync.dma_start(out=xt[:], in_=xT_p[128 * kd:128 * (kd + 1), 0:512])
                xts0.append(xt)
            nc.sync.dma_start(out=cos4[:], in_=cos4_p[:])
            nc.sync.dma_start(out=sin4[:], in_=sin4_p[:])
            nc.sync.dma_start(out=masks[:], in_=mask_p[:])
            nc.sync.dma_start(out=ident[:], in_=ident_p[:])
            nc.vector.memset(V[:], 1.0)

            aout_c = {}   # (c, hp) -> per-chunk fp16 attention-output tile

            def outproj_unit(cc, sb, dg):
                # one (s-block, 1024-wide D group): 8 matmuls + 2 evacs + 1 DMA
                ost = opool.tile([128, 1024], F32, tag='ost')
                for i, dc in enumerate((2 * dg, 2 * dg + 1)):
                    po = accp.tile([128, 512], F32, tag='acc', name=f'po{sb}_{dc}')
                    for hc in range(4):
                        nc.tensor.matmul(po[:], aout_c[(cc, hc)][:, 128 * (sb - 4 * cc):128 * (sb - 4 * cc) + 128],
                                         WoT[:, hc * D + 512 * dc: hc * D + 512 * (dc + 1)],
                                         start=(hc == 0), stop=(hc == 3))
                    if dc % 2 == 0:
                        nc.scalar.copy(ost[:, 512 * i:512 * (i + 1)], po[:])
                    else:
                        nc.vector.tensor_copy(ost[:, 512 * i:512 * (i + 1)], po[:])
                nc.sync.dma_start(out=out_p[128 * sb:128 * (sb + 1), 1024 * dg:1024 * (dg + 1)],
                                  in_=ost[:])

            qc_c = {}    # chunk -> list of 4 q tiles
            xts_c = {0: xts0}

            def emit_xt_dmas(cc):
                if cc in xts_c or cc >= SC:
                    return
                lst = []
                for kd in range(KD):
                    xt = xpool.tile([128, 512], F32R, tag='xt')
                    nc.sync.dma_start(out=xt[:],
                                      in_=xT_p[128 * kd:128 * (kd + 1), 512 * cc:512 * (cc + 1)])
                    lst.append(xt)
                xts_c[cc] = lst

            def proj_unit(cc, m):
                # one projection output block (128 rows) + its epilogue
                cs = slice(512 * cc, 512 * (cc + 1))
                xts = xts_c[cc]
                ps = accp.tile([128, 512], F32, tag='acc', name=f'ps{cc}_{m}')
                for kd in range(KD):
                    nc.tensor.matmul(ps[:], WT[:, kd * 768 + 128 * m: kd * 768 + 128 * (m + 1)],
                                     xts[kd][:], start=(kd == 0), stop=(kd == KD - 1))
                if m < 5:
                    # RoPE: out = ps*cos4 + swap32(ps)*sin4 (sign baked in sin4)
                    t1 = tpool.tile([128, 512], F32, tag='t1')
                    t2 = tpool.tile([128, 512], F32, tag='t2')
                    nc.vector.tensor_mul(t1[:], ps[:], cos4[:, cs])
                    for g in range(2):
                        b0 = 64 * g
                        nc.vector.tensor_mul(t2[b0:b0 + 32, :], ps[b0 + 32:b0 + 64, :],
                                             sin4[b0:b0 + 32, cs])
                        nc.vector.tensor_mul(t2[b0 + 32:b0 + 64, :], ps[b0:b0 + 32, :],
                                             sin4[b0 + 32:b0 + 64, cs])
                    tgt = qc_c[cc][m][:] if m < 4 else k4[:, cs]
                    nc.vector.tensor_add(tgt, t1[:], t2[:])
                    if m == 4:
                        # kT replication: rep0=[kv0|kv0], rep1=[kv1|kv1]
                        nc.vector.tensor_add(kswap[0:64, cs], t1[64:128, :], t2[64:128, :])
                        nc.vector.tensor_add(kswap[64:128, cs], t1[0:64, :], t2[0:64, :])
                else:
                    vT = vpool.tile([128, 512], F32, tag='vT')
                    nc.scalar.copy(vT[:], ps[:])
                    # V natural (bf16): per j [ones|v0|ones|v1|ones] (5x64)
                    for j in range(4 * cc, 4 * cc + 4):
                        jj = j - 4 * cc
                        pt = accp.tile([128, 128], F32, tag='acc', name=f'pt{j}')
                        nc.tensor.transpose(pt[:], vT[:, 128 * jj:128 * jj + 128], ident[:])
                        nc.vector.tensor_copy(V[:, 320 * j + 64:320 * j + 128], pt[:, 0:64])
                        nc.vector.tensor_copy(V[:, 320 * j + 192:320 * j + 256], pt[:, 64:128])

            def emit_proj(cc):
                emit_xt_dmas(cc)
                qc_c[cc] = [qpool.tile([128, 512], BF16, tag='qc', name=f'qc{cc}_{m}')
                            for m in range(4)]
                for m in (4, 5, 0, 1, 2, 3):
                    proj_unit(cc, m)

            emit_proj(0)
            for hc in range(4):
                nc.sync.dma_start(out=WoT[:, hc * D:(hc + 1) * D],
                                  in_=WoT_p[128 * hc:128 * (hc + 1), :])

            for c in range(SC):
                # filler units spread through attention(c):
                # outproj of chunk c-1, then projection blocks of chunk c+1
                fillers = []
                if c >= 1:
                    for sb in range(4 * (c - 1), 4 * c):
                        for dg in range(2):
                            fillers.append((outproj_unit, (c - 1, sb, dg)))
                if c + 1 < SC:
                    emit_xt_dmas(c + 1)
                    qc_c[c + 1] = [qpool.tile([128, 512], BF16, tag='qc', name=f'qc{c+1}_{m}')
                                   for m in range(4)]
                    for m in (4, 5, 0, 1, 2, 3):
                        fillers.append((proj_unit, (c + 1, m)))
                nj = 4 * c + 4
                npoints = 4 * (nj - 1)
                nfil = len(fillers)
                thresholds = [round((f + 1) * npoints / (nfil + 1)) for f in range(nfil)]
                point = 0
                qc = qc_c[c]

                for hp in range(4):
                    kv = hp // 2
                    oAB = pop.tile([128, 1024], F32, tag='oab')
                    atas = {}

                    def scores(j):
                        sct = scp.tile([128, 1024], F32, tag='sc', name=f'sc{hp}_{c}_{j}')
                        if kv == 0:
                            kA = k4[0:64, 128 * j:128 * (j + 1)]
                            kB = kswap[64:128, 128 * j:128 * (j + 1)]
                        else:
                            kA = kswap[0:64, 128 * j:128 * (j + 1)]
                            kB = k4[64:128, 128 * j:128 * (j + 1)]
                        nc.tensor.matmul(sct[:, 0:512], kA, qc[hp][0:64, :], start=True, stop=True)
                        nc.tensor.matmul(sct[:, 512:1024], kB, qc[hp][64:128, :], start=True, stop=True)
                        ata = atpool.tile([128, 1024], BF16, tag='at', name=f'at{hp}_{c}_{j}')
                        nc.scalar.activation(ata[:], sct[:], mybir.ActivationFunctionType.Exp,
                                             scale=SCALE)
                        d = j - 4 * c
                        if 0 <= d <= 3:
                            nc.vector.tensor_mul(ata[:, 0:512], ata[:, 0:512],
                                                 masks[:, 1024 * d:1024 * d + 512])
                            nc.gpsimd.tensor_mul(ata[:, 512:1024], ata[:, 512:1024],
                                                 masks[:, 1024 * d + 512:1024 * (d + 1)])
                        atas[j] = ata

                    def attnv(j):
                        # A slice [v|ones] -> rows 0:64 data, 64:128 denom
                        # B slice [ones|v] -> rows 0:64 denom, 64:128 data
                        ata = atas.pop(j)
                        base = 320 * j + 128 * kv
                        nc.tensor.matmul(oAB[:, 0:512], V[:, base + 64:base + 192],
                                         ata[:, 0:512], start=(j == 0), stop=(j == nj - 1))
                        nc.tensor.matmul(oAB[:, 512:1024], V[:, base:base + 128],
                                         ata[:, 512:1024], start=(j == 0), stop=(j == nj - 1))

                    scores(0)
                    for j in range(1, nj):
                        scores(j)
                        attnv(j - 1)
                        point += 1
                        while fillers and thresholds and point >= thresholds[0]:
                            thresholds.pop(0)
                            fn, args = fillers.pop(0)
                            fn(*args)
                    attnv(nj - 1)

                    # normalize: evacuate PSUM into base-0 data/denom tiles,
                    # approx recip on the denom tile, two base-aligned muls
                    dsw = spool.tile([64, 1024], F32, tag='dsw')
                    nc.scalar.copy(dsw[:, 0:512], oAB[64:128, 0:512])
                    nc.scalar.copy(dsw[:, 512:1024], oAB[0:64, 512:1024])
                    sbD = spool.tile([64, 1024], F32, tag='sbd')
                    nc.vector.tensor_copy(sbD[:, 0:512], oAB[0:64, 0:512])
                    nc.vector.tensor_copy(sbD[:, 512:1024], oAB[64:128, 512:1024])
                    rr = spool.tile([64, 1024], F32, tag='rr')
                    nc.vector.reciprocal_approx_fast(rr[:], dsw[:])
                    ao = aopool.tile([128, 512], F16, tag='ao', name=f'ao{c}_{hp}')
                    aout_c[(c, hp)] = ao
                    nc.vector.tensor_mul(ao[0:64, :], sbD[:, 0:512], rr[:, 0:512])
                    nc.vector.tensor_mul(ao[64:128, :], sbD[:, 512:1024], rr[:, 512:1024])

                for fn, args in fillers:
                    fn(*args)

            for sb in range(12, 16):
                for dg in range(2):
                    outproj_unit(3, sb, dg)
    nc.compile()
    return nc


_PERM = np.concatenate([np.arange(0, DH, 2), np.arange(1, DH, 2)])


def _prep_core(x, Wq, Wk, Wv, Wo, cos, sin, b, hg):
    xT = np.ascontiguousarray(x[b].T.astype(np.float32))
    # q heads 8hg..8hg+7 permuted, kv heads 2hg,2hg+1 (k permuted, v natural)
    wq = Wq.reshape(H, DH, D)[8 * hg:8 * hg + 8][:, _PERM, :].reshape(512, D)
    wk = Wk.reshape(KVH, DH, D)[2 * hg:2 * hg + 2][:, _PERM, :].reshape(128, D)
    wv = Wv.reshape(KVH, DH, D)[2 * hg:2 * hg + 2].reshape(128, D)
    WT = np.ascontiguousarray(np.concatenate([wq, wk, wv], 0).T.astype(np.float32))
    WoT = np.ascontiguousarray(Wo[:, 512 * hg:512 * (hg + 1)].T.astype(np.float16))
    cosT = np.ascontiguousarray(cos.T.astype(np.float32))          # (32, S)
    sinT = np.ascontiguousarray(sin.T.astype(np.float32))
    cos4 = np.tile(cosT, (4, 1)).astype(np.float16)
    sin4 = np.concatenate([-sinT, sinT, -sinT, sinT], 0).astype(np.float16)
    mask = np.zeros((128, 4 * 1024), dtype=np.float64)
    for dd in range(4):
        blk = (128 * dd + np.arange(128)[:, None]) <= np.arange(512)[None, :]
        mask[:, 1024 * dd:1024 * dd + 512] = blk
        mask[:, 1024 * dd + 512:1024 * (dd + 1)] = blk
    return {'xT': xT, 'WT': WT, 'WoT': WoT, 'cos4': cos4, 'sin4': sin4,
            'mask': mask.astype(ml_dtypes.bfloat16),
            'ident': np.eye(128, dtype=np.float32)}


def _run(inputs, trace=False, tmpdir=None):
    if 'nc' not in _cache:
        _cache['nc'] = build()
    in_maps = [_prep_core(inputs['x'], inputs['Wq'], inputs['Wk'], inputs['Wv'],
                          inputs['Wo'], inputs['cos'], inputs['sin'], c // 4, c % 4)
               for c in range(8)]
    res = run_bass_kernel_spmd(_cache['nc'], in_maps, core_ids=list(range(8)),
                               trace=trace, tmpdir=tmpdir)
    parts = [res.results[c]['out'] for c in range(8)]
    out = np.stack([parts[0] + parts[1] + parts[2] + parts[3],
                    parts[4] + parts[5] + parts[6] + parts[7]], 0)
    return out.astype(np.float32), res


def kernel(**inputs):
    out, _ = _run(inputs, trace=False)
    return out


# revision 34
# speedup vs baseline: 1.0880x; 1.0135x over previous
"""GQA kernel for trn2: 8 NeuronCores, SPMD (b in {0,1} x 4 head-groups).

Per core (b, hg): 8 q-heads (8hg..8hg+7), 2 kv-heads (2hg, 2hg+1).
c-major software pipeline: per 512-wide q chunk c emit
  proj(c) -> RoPE -> V-build -> attention(c) with outproj(c-1) interleaved
so PE never sees a phase barrier (keeps HAM un-throttled).
V blocks are [v0|ones64|v1] so the attn@V matmul also produces the softmax
denominator replicated across 64 partitions (normalize = recip + 2 muls,
no partition broadcast). f32r proj, bf16 scores/attn@V, fp16 outproj.
Host preps transposed/permuted weights; partial outputs summed on host
(row-parallel Wo all-reduce done during unshard).
"""
import numpy as np
import ml_dtypes
import concourse.bass as bass
import concourse.mybir as mybir
from concourse import tile, bacc
from concourse.bass_utils import run_bass_kernel_spmd

B, S, D = 2, 2048, 2048
H, KVH, DH = 32, 8, 64
SCALE = DH ** -0.5
SC = 4          # Sq chunks of 512
KD = 16         # D contraction chunks of 128
NJ = 16         # Sk blocks of 128
F32R = mybir.dt.float32r
F32 = mybir.dt.float32
BF16 = mybir.dt.bfloat16
F16 = mybir.dt.float16

_cache = {}


def build():
    nc = bacc.Bacc('TRN2', target_bir_lowering=False, debug=False, num_devices=8)
    xT_p = nc.declare_dram_parameter('xT', [D, S], F32R, isOutput=False)
    WT_p = nc.declare_dram_parameter('WT', [D, 768], F32R, isOutput=False)
    WoT_p = nc.declare_dram_parameter('WoT', [512, D], F16, isOutput=False)
    cos4_p = nc.declare_dram_parameter('cos4', [128, S], F16, isOutput=False)
    sin4_p = nc.declare_dram_parameter('sin4', [128, S], F16, isOutput=False)
    mask_p = nc.declare_dram_parameter('mask', [128, 4 * 1024], BF16, isOutput=False)
    ident_p = nc.declare_dram_parameter('ident', [128, 128], F32, isOutput=False)
    out_p = nc.declare_dram_parameter('out', [S, D], F32, isOutput=True)

    with tile.TileContext(nc) as tc:
        with tc.tile_pool(name='w', bufs=1) as wpool, \
             tc.tile_pool(name='x', bufs=20) as xpool, \
             tc.tile_pool(name='q', bufs=8) as qpool, \
             tc.tile_pool(name='ao', bufs=8) as aopool, \
             tc.tile_pool(name='v', bufs=2) as vpool, \
             tc.tile_pool(name='t', bufs=2) as tpool, \
             tc.tile_pool(name='at', bufs=4) as atpool, \
             tc.tile_pool(name='s', bufs=2) as spool, \
             tc.tile_pool(name='o', bufs=2) as opool, \
             tc.tile_pool(name='acc', bufs=2, space='PSUM') as accp, \
             tc.tile_pool(name='sc', bufs=2, space='PSUM') as scp, \
             tc.tile_pool(name='po', bufs=1, space='PSUM') as pop:

            WT = wpool.tile([128, KD * 768], F32R, tag='WT')
            cos4 = wpool.tile([128, S], F16, tag='cos4')
            sin4 = wpool.tile([128, S], F16, tag='sin4')
            masks = wpool.tile([128, 4 * 1024], BF16, tag='masks')
            ident = wpool.tile([128, 128], F32, tag='ident')
            k4 = wpool.tile([128, S], BF16, tag='k4')
            kswap = wpool.tile([128, S], BF16, tag='kswap')
            V = wpool.tile([128, NJ * 256], BF16, tag='V')
            WoT = wpool.tile([128, 4 * D], F16, tag='WoT')

            # startup: interleave WT and first-chunk x DMAs so kd=0 lands fast
            xts0 = []
            for kd in range(KD):
                nc.sync.dma_start(out=WT[:, kd * 768:(kd + 1) * 768],
                                  in_=WT_p[128 * kd:128 * (kd + 1), :])
                xt = xpool.tile([128, 512], F32R, tag='xt')
                nc.sync.dma_start(out=xt[:], in_=xT_p[128 * kd:128 * (kd + 1), 0:512])
                xts0.append(xt)
            nc.sync.dma_start(out=cos4[:], in_=cos4_p[:])
            nc.sync.dma_start(out=sin4[:], in_=sin4_p[:])
            nc.sync.dma_start(out=masks[:], in_=mask_p[:])
            nc.sync.dma_start(out=ident[:], in_=ident_p[:])
            nc.vector.memset(V[:], 1.0)

            aout_c = {}   # (c, hp) -> per-chunk fp16 attention-output tile

            def outproj_unit(cc, sb, dg):
                # one (s-block, 1024-wide D group): hc-major so the aout block
                # stays the PE stationary across each dc pair (ldweights reuse)
                ost = opool.tile([128, 1024], F32, tag='ost')
                dc0, dc1 = 2 * dg, 2 * dg + 1
                po0 = accp.tile([128, 512], F32, tag='acc', name=f'po{sb}_{dc0}')
                po1 = accp.tile([128, 512], F32, tag='acc', name=f'po{sb}_{dc1}')
                for hc in range(4):
                    lhs = aout_c[(cc, hc)][:, 128 * (sb - 4 * cc):128 * (sb - 4 * cc) + 128]
                    nc.tensor.matmul(po0[:], lhs, WoT[:, hc * D + 512 * dc0: hc * D + 512 * (dc0 + 1)],
                                     start=(hc == 0), stop=(hc == 3))
                    nc.tensor.matmul(po1[:], lhs, WoT[:, hc * D + 512 * dc1: hc * D + 512 * (dc1 + 1)],
                                     start=(hc == 0), stop=(hc == 3))
                nc.scalar.copy(ost[:, 0:512], po0[:])
                nc.vector.tensor_copy(ost[:, 512:1024], po1[:])
                nc.sync.dma_start(out=out_p[128 * sb:128 * (sb + 1), 1024 * dg:1024 * (dg + 1)],
                                  in_=ost[:])

            qc_c = {}    # chunk -> list of 4 q tiles
            xts_c = {0: xts0}

            def emit_xt_dmas(cc):
                if cc in xts_c or cc >= SC:
                    return
                lst = []
                for kd in range(KD):
                    xt = xpool.tile([128, 512], F32R, tag='xt')
                    nc.sync.dma_start(out=xt[:],
                                      in_=xT_p[128 * kd:128 * (kd + 1), 512 * cc:512 * (cc + 1)])
                    lst.append(xt)
                xts_c[cc] = lst

            def proj_unit(cc, m):
                # one projection output block (128 rows) + its epilogue
                cs = slice(512 * cc, 512 * (cc + 1))
                xts = xts_c[cc]
                ps = accp.tile([128, 512], F32, tag='acc', name=f'ps{cc}_{m}')
                for kd in range(KD):
                    nc.tensor.matmul(ps[:], WT[:, kd * 768 + 128 * m: kd * 768 + 128 * (m + 1)],
                                     xts[kd][:], start=(kd == 0), stop=(kd == KD - 1))
                if m < 5:
                    # RoPE: out = ps*cos4 + swap32(ps)*sin4 (sign baked in sin4)
                    t1 = tpool.tile([128, 512], F32, tag='t1')
                    t2 = tpool.tile([128, 512], F32, tag='t2')
                    nc.vector.tensor_mul(t1[:], ps[:], cos4[:, cs])
                    for g in range(2):
                        b0 = 64 * g
                        nc.vector.tensor_mul(t2[b0:b0 + 32, :], ps[b0 + 32:b0 + 64, :],
                                             sin4[b0:b0 + 32, cs])
                        nc.vector.tensor_mul(t2[b0 + 32:b0 + 64, :], ps[b0:b0 + 32, :],
                                             sin4[b0 + 32:b0 + 64, cs])
                    tgt = qc_c[cc][m][:] if m < 4 else k4[:, cs]
                    nc.vector.tensor_add(tgt, t1[:], t2[:])
                    if m == 4:
                        # kT replication: rep0=[kv0|kv0], rep1=[kv1|kv1]
                        nc.vector.tensor_add(kswap[0:64, cs], t1[64:128, :], t2[64:128, :])
                        nc.vector.tensor_add(kswap[64:128, cs], t1[0:64, :], t2[0:64, :])
                else:
                    vT = vpool.tile([128, 512], F32, tag='vT')
                    nc.scalar.copy(vT[:], ps[:])
                    # V natural (bf16): per j [v0|ones|v1|ones] (4x64)
                    for j in range(4 * cc, 4 * cc + 4):
                        jj = j - 4 * cc
                        pt = accp.tile([128, 128], F32, tag='acc', name=f'pt{j}')
                        nc.tensor.transpose(pt[:], vT[:, 128 * jj:128 * jj + 128], ident[:])
                        nc.vector.tensor_copy(V[:, 256 * j:256 * j + 64], pt[:, 0:64])
                        nc.vector.tensor_copy(V[:, 256 * j + 128:256 * j + 192], pt[:, 64:128])

            def emit_proj(cc):
                emit_xt_dmas(cc)
                qc_c[cc] = [qpool.tile([128, 512], BF16, tag='qc', name=f'qc{cc}_{m}')
                            for m in range(4)]
                for m in (4, 5, 0, 1, 2, 3):
                    proj_unit(cc, m)

            emit_proj(0)
            for hc in range(4):
                nc.sync.dma_start(out=WoT[:, hc * D:(hc + 1) * D],
                                  in_=WoT_p[128 * hc:128 * (hc + 1), :])

            for c in range(SC):
                # filler units spread through attention(c):
                # outproj of chunk c-1, then projection blocks of chunk c+1
                fillers = []
                if c >= 1:
                    for sb in range(4 * (c - 1), 4 * c):
                        for dg in range(2):
                            fillers.append((outproj_unit, (c - 1, sb, dg)))
                if c + 1 < SC:
                    emit_xt_dmas(c + 1)
                    qc_c[c + 1] = [qpool.tile([128, 512], BF16, tag='qc', name=f'qc{c+1}_{m}')
                                   for m in range(4)]
                    for m in (4, 5, 0, 1, 2, 3):
                        fillers.append((proj_unit, (c + 1, m)))
                nj = 4 * c + 4
                npoints = 4 * (nj - 1)
                nfil = len(fillers)
                thresholds = [round((f + 1) * npoints / (nfil + 1)) for f in range(nfil)]
                point = 0
                qc = qc_c[c]

                for hp in range(4):
                    kv = hp // 2
                    oAB = pop.tile([128, 1024], F32, tag='oab')
                    atas = {}

                    def scores(j):
                        sct = scp.tile([128, 1024], F32, tag='sc', name=f'sc{hp}_{c}_{j}')
                        if kv == 0:
                            kA = k4[0:64, 128 * j:128 * (j + 1)]
                            kB = kswap[64:128, 128 * j:128 * (j + 1)]
                        else:
                            kA = kswap[0:64, 128 * j:128 * (j + 1)]
                            kB = k4[64:128, 128 * j:128 * (j + 1)]
                        nc.tensor.matmul(sct[:, 0:512], kA, qc[hp][0:64, :], start=True, stop=True)
                        nc.tensor.matmul(sct[:, 512:1024], kB, qc[hp][64:128, :], start=True, stop=True)
                        ata = atpool.tile([128, 1024], BF16, tag='at', name=f'at{hp}_{c}_{j}')
                        nc.scalar.activation(ata[:], sct[:], mybir.ActivationFunctionType.Exp,
                                             scale=SCALE)
                        d = j - 4 * c
                        if 0 <= d <= 3:
                            nc.vector.tensor_mul(ata[:, 0:512], ata[:, 0:512],
                                                 masks[:, 1024 * d:1024 * d + 512])
                            nc.gpsimd.tensor_mul(ata[:, 512:1024], ata[:, 512:1024],
                                                 masks[:, 1024 * d + 512:1024 * (d + 1)])
                        atas[j] = ata

                    def attnv(j):
                        # both heads share the kv head -> identical [v|ones]
                        # stationary for A and B (rows 0:64 data, 64:128 denom)
                        ata = atas.pop(j)
                        vs = V[:, 256 * j + 128 * kv:256 * j + 128 * kv + 128]
                        nc.tensor.matmul(oAB[:, 0:512], vs,
                                         ata[:, 0:512], start=(j == 0), stop=(j == nj - 1))
                        nc.tensor.matmul(oAB[:, 512:1024], vs,
                                         ata[:, 512:1024], start=(j == 0), stop=(j == nj - 1))

                    scores(0)
                    for j in range(1, nj):
                        scores(j)
                        attnv(j - 1)
                        point += 1
                        while fillers and thresholds and point >= thresholds[0]:
                            thresholds.pop(0)
                            fn, args = fillers.pop(0)
                            fn(*args)
                    attnv(nj - 1)

                    # normalize: evacuate PSUM into base-0 data/denom tiles,
                    # approx recip on the denom tile, two base-aligned muls
                    dsw = spool.tile([64, 1024], F32, tag='dsw')
                    nc.scalar.copy(dsw[:], oAB[64:128, :])
                    sbD = spool.tile([64, 1024], F32, tag='sbd')
                    nc.vector.tensor_copy(sbD[:], oAB[0:64, :])
                    rr = spool.tile([64, 1024], F32, tag='rr')
                    nc.vector.reciprocal_approx_fast(rr[:], dsw[:])
                    ao = aopool.tile([128, 512], F16, tag='ao', name=f'ao{c}_{hp}')
                    aout_c[(c, hp)] = ao
                    nc.vector.tensor_mul(ao[0:64, :], sbD[:, 0:512], rr[:, 0:512])
                    nc.vector.tensor_mul(ao[64:128, :], sbD[:, 512:1024], rr[:, 512:1024])

                for fn, args in fillers:
                    fn(*args)

            for sb in range(12, 16):
                for dg in range(2):
                    outproj_unit(3, sb, dg)
    nc.compile()
    return nc


_PERM = np.concatenate([np.arange(0, DH, 2), np.arange(1, DH, 2)])


def _prep_core(x, Wq, Wk, Wv, Wo, cos, sin, b, hg):
    xT = np.ascontiguousarray(x[b].T.astype(np.float32))
    # q heads 8hg..8hg+7 permuted, kv heads 2hg,2hg+1 (k permuted, v natural)
    wq = Wq.reshape(H, DH, D)[8 * hg:8 * hg + 8][:, _PERM, :].reshape(512, D)
    wk = Wk.reshape(KVH, DH, D)[2 * hg:2 * hg + 2][:, _PERM, :].reshape(128, D)
    wv = Wv.reshape(KVH, DH, D)[2 * hg:2 * hg + 2].reshape(128, D)
    WT = np.ascontiguousarray(np.concatenate([wq, wk, wv], 0).T.astype(np.float32))
    WoT = np.ascontiguousarray(Wo[:, 512 * hg:512 * (hg + 1)].T.astype(np.float16))
    cosT = np.ascontiguousarray(cos.T.astype(np.float32))          # (32, S)
    sinT = np.ascontiguousarray(sin.T.astype(np.float32))
    cos4 = np.tile(cosT, (4, 1)).astype(np.float16)
    sin4 = np.concatenate([-sinT, sinT, -sinT, sinT], 0).astype(np.float16)
    mask = np.zeros((128, 4 * 1024), dtype=np.float64)
    for dd in range(4):
        blk = (128 * dd + np.arange(128)[:, None]) <= np.arange(512)[None, :]
        mask[:, 1024 * dd:1024 * dd + 512] = blk
        mask[:, 1024 * dd + 512:1024 * (dd + 1)] = blk
    return {'xT': xT, 'WT': WT, 'WoT': WoT, 'cos4': cos4, 'sin4': sin4,
            'mask': mask.astype(ml_dtypes.bfloat16),
            'ident': np.eye(128, dtype=np.float32)}


def _run(inputs, trace=False, tmpdir=None):
    if 'nc' not in _cache:
        _cache['nc'] = build()
    in_maps = [_prep_core(inputs['x'], inputs['Wq'], inputs['Wk'], inputs['Wv'],
                          inputs['Wo'], inputs['cos'], inputs['sin'], c // 4, c % 4)
               for c in range(8)]
    res = run_bass_kernel_spmd(_cache['nc'], in_maps, core_ids=list(range(8)),
                               trace=trace, tmpdir=tmpdir)
    parts = [res.results[c]['out'] for c in range(8)]
    out = np.stack([parts[0] + parts[1] + parts[2] + parts[3],
                    parts[4] + parts[5] + parts[6] + parts[7]], 0)
    return out.astype(np.float32), res


def kernel(**inputs):
    out, _ = _run(inputs, trace=False)
    return out
